# revision 1
# baseline (speedup 1.0000x reference)
"""ChebyshevGCN (K=3) on 8 TRN2 NeuronCores.

Strategy (dst-sharded SpMM via one-hot matmuls):
  - Nodes dst-sharded across 8 cores (12544 padded rows each); small weights
    replicated. Tables g1 = dis*x and g2 = -dis^2*S (fp16) are AllGathered so
    every core gathers feature rows locally (the "halo exchange").
  - Per-edge feature rows are fetched with dma_gather (int16 idx, 4 SWDGE
    queues, 4 sub-tables of 25088 rows so indices fit int16).
  - Scatter-add to dst is a one-hot matmul: onehot[e, dstoff] = w_e built by a
    fused DVE tensor_scalar(is_equal, mult) vs an iota tile; PE accumulates
    [128dst x 128f] windows in PSUM; quarters accumulate into an SBUF y_acc.
  - Chebyshev algebra: out = x@(W0-W2) + Tx1@W1 + (-2 dis*S2)@W2 with
    Tx1 = -dis*S1, so Tx2 is never materialized.
  - Dense epilogue in filter-major form: psum = W'^T @ hT tiles (hT via fp16
    DMA-transpose), relu(+b_cheb) on ACT, then a [filt]x[filt,1] matmul with
    W_lin. Degree/normalization (deg, dis=rsqrt(deg)) computed on device.
"""
import sys
import math
import numpy as np

if "/opt/trn_rl_repo" not in sys.path:
    sys.path.insert(0, "/opt/trn_rl_repo")

import concourse.bass as bass  # noqa: F401
import concourse.mybir as mybir
import concourse.tile as tile
from concourse import bacc, bass_utils

F = 128
GCH = 32          # chunks (of 128 edges) per dma_gather call
TRACE = [False]   # test.py flips this to get exec_time_ns
LAST_EXEC_NS = [None]


def _ceil(a, b):
    return (a + b - 1) // b


def _plan(x, edge_index, edge_weight, n_cores=8):
    N = x.shape[0]
    S_LOG = _ceil(N, n_cores)
    SHARD = _ceil(S_LOG, 128) * 128
    NTAB = n_cores * SHARD
    QT = NTAB // 4
    assert QT <= 32768
    NW = SHARD // 128

    src = np.asarray(edge_index[0], dtype=np.int64)
    dst = np.asarray(edge_index[1], dtype=np.int64)
    w = np.asarray(edge_weight, dtype=np.float32)

    owner = dst // S_LOG
    dl = dst - owner * S_LOG
    srow = (src // S_LOG) * SHARD + (src % S_LOG)
    q_of = srow // QT
    qidx = (srow % QT).astype(np.int16)
    win = dl // 128
    doff = (dl % 128).astype(np.float32)

    # per-core run counts -> shared K[q][w]
    per_core = []
    cnts = np.zeros((n_cores, 4 * NW), np.int64)
    for c in range(n_cores):
        sel = np.nonzero(owner == c)[0]
        qc, wc = q_of[sel], win[sel]
        order = np.lexsort((wc, qc))
        sel = sel[order]
        run = q_of[sel] * NW + win[sel]
        cnts[c] = np.bincount(run, minlength=4 * NW)
        per_core.append((sel, run))
    K = _ceil(cnts.max(axis=0), 128).reshape(4, NW)          # chunks per run
    K = np.maximum(K, 1)
    TOTCH = int(K.sum())
    runK = K.reshape(-1)
    run_base = np.concatenate([[0], np.cumsum(runK)])[:-1]    # chunk offset/run
    CQ = K.sum(axis=1)                                        # chunks/quarter
    cbase = np.concatenate([[0], np.cumsum(CQ)])[:-1]

    # gather-call metadata (shared): per quarter split CQ into GCH-chunk calls
    call_meta = []                                            # (cstart, nch)
    for q in range(4):
        left, cs = int(CQ[q]), int(cbase[q])
        while left > 0:
            n = min(GCH, left)
            call_meta.append((cs, n))
            cs += n
            left -= n
    NCALLS = len(call_meta)

    # out-degree padding for deg reduce
    deg_cnt = np.bincount(src, minlength=N)
    PAD = max(8, _ceil(int(deg_cnt.max()), 8) * 8)

    # per-core arrays
    in_maps = []
    for c in range(n_cores):
        sel, run = per_core[c]
        starts = np.concatenate([[0], np.cumsum(cnts[c])])[:-1]
        rank = np.arange(len(sel)) - starts[run]
        slot = run_base[run] * 128 + rank
        E_s = TOTCH * 128
        qidx_s = np.zeros(E_s, np.int16)
        doff_s = np.full(E_s, 999.0, np.float32)
        w_s = np.zeros(E_s, np.float32)
        qidx_s[slot] = qidx[sel]
        doff_s[slot] = doff[sel]
        w_s[slot] = w[sel]
        dstw = np.empty((128, 2 * TOTCH), np.float32)
        dstw[:, 0::2] = doff_s.reshape(TOTCH, 128).T
        dstw[:, 1::2] = w_s.reshape(TOTCH, 128).T
        idxs = np.zeros((NCALLS, 128, GCH * 8), np.int16)
        for i, (cs, n) in enumerate(call_meta):
            ids = qidx_s[cs * 128:(cs + n) * 128]
            wrap = ids.reshape(n * 8, 16).T                   # [16, n*8]
            idxs[i, :, :n * 8] = np.tile(wrap, (8, 1))
        # w_pad for deg (out-edges of own shard nodes)
        sel2 = np.nonzero(src // S_LOG == c)[0]
        loc = (src[sel2] - c * S_LOG).astype(np.int64)
        o2 = np.argsort(loc, kind="stable")
        sel2, loc = sel2[o2], loc[o2]
        c2 = np.bincount(loc, minlength=S_LOG)
        st2 = np.concatenate([[0], np.cumsum(c2)])[:-1]
        rk2 = np.arange(len(sel2)) - st2[loc]
        wpad = np.zeros((NW, 128, PAD), np.float32)
        wpad[loc // 128, loc % 128, rk2] = w[sel2]
        xs = np.zeros((SHARD, F), np.float32)
        n0, n1 = c * S_LOG, min((c + 1) * S_LOG, N)
        xs[: n1 - n0] = np.asarray(x[n0:n1], np.float32)
        in_maps.append({
            "x32": xs, "x16": xs.astype(np.float16), "wpad": wpad,
            "dstw": dstw, "idxs": idxs,
        })
    shape = dict(N=N, S_LOG=S_LOG, SHARD=SHARD, NTAB=NTAB, QT=QT, NW=NW,
                 PAD=PAD, TOTCH=TOTCH, NCALLS=NCALLS, K=K,
                 call_meta=call_meta, cbase=cbase, n_cores=n_cores)
    return shape, in_maps


def _build(p, b_lin_val):
    n_cores, SHARD, NTAB, QT, NW, PAD, TOTCH, NCALLS = (
        p["n_cores"], p["SHARD"], p["NTAB"], p["QT"], p["NW"], p["PAD"],
        p["TOTCH"], p["NCALLS"])
    K, call_meta = p["K"], p["call_meta"]
    f32, f16, i16, i32 = (mybir.dt.float32, mybir.dt.float16,
                          mybir.dt.int16, mybir.dt.int32)
    Alu, Act = mybir.AluOpType, mybir.ActivationFunctionType

    nc = bacc.Bacc("TRN2", target_bir_lowering=False, debug=False,
                   num_devices=n_cores, num_swdge_queues=4)
    x32 = nc.dram_tensor("x32", [SHARD, F], f32, kind="ExternalInput")
    x16 = nc.dram_tensor("x16", [SHARD, F], f16, kind="ExternalInput")
    wpad = nc.dram_tensor("wpad", [NW, 128, PAD], f32, kind="ExternalInput")
    dstw = nc.dram_tensor("dstw", [128, 2 * TOTCH], f32, kind="ExternalInput")
    idxs = nc.dram_tensor("idxs", [NCALLS, 128, GCH * 8], i16,
                          kind="ExternalInput")
    wch = nc.dram_tensor("wch", [3, 128, 128], f32, kind="ExternalInput")
    bch = nc.dram_tensor("bch", [128, 1], f32, kind="ExternalInput")
    wlin = nc.dram_tensor("wlin", [128, 1], f32, kind="ExternalInput")
    out = nc.dram_tensor("out", [SHARD, 1], f32, kind="ExternalOutput")

    ag1_in = nc.dram_tensor("ag1_in", [SHARD, F], f16, kind="Internal")
    g1_full = nc.dram_tensor("g1_full", [NTAB, F], f16, kind="Internal",
                             addr_space="Shared")
    ag2_in = nc.dram_tensor("ag2_in", [SHARD, F], f16, kind="Internal")
    g2_full = nc.dram_tensor("g2_full", [NTAB, F], f16, kind="Internal",
                             addr_space="Shared")
    tx1s = nc.dram_tensor("tx1s", [SHARD, F], f16, kind="Internal")
    s2s = nc.dram_tensor("s2s", [SHARD, F], f16, kind="Internal")
    rg = [list(range(n_cores))]

    with tile.TileContext(nc) as tc:
        with tc.tile_pool(name="pp", bufs=1) as pp, \
             tc.tile_pool(name="sp", bufs=3) as sp, \
             tc.tile_pool(name="gst", bufs=4) as gp, \
             tc.tile_pool(name="oh", bufs=6) as ohp, \
             tc.tile_pool(name="psA", bufs=3, space="PSUM") as psA, \
             tc.tile_pool(name="psB", bufs=2, space="PSUM") as psB, \
             tc.tile_pool(name="psC", bufs=2, space="PSUM") as psC:

            # ---- prep: streams, weights, iota -------------------------------
            dstw_t = pp.tile([128, 2 * TOTCH], f32)
            nc.sync.dma_start(dstw_t[:], dstw[:, :])
            iota_i = pp.tile([128, 128], i32)
            nc.gpsimd.iota(iota_i[:], pattern=[[1, 128]], base=0,
                           channel_multiplier=0)
            iota_f = pp.tile([128, 128], f32)
            nc.vector.tensor_copy(iota_f[:], iota_i[:])
            w0t = pp.tile([128, 128], f32)
            w2t = pp.tile([128, 128], f32)
            nc.sync.dma_start(w0t[:], wch[0, :, :])
            nc.sync.dma_start(w2t[:], wch[2, :, :])
            w02f = pp.tile([128, 128], f16)
            nc.vector.tensor_tensor(out=w02f[:], in0=w0t[:], in1=w2t[:],
                                    op=Alu.subtract)
            w1f = pp.tile([128, 128], f16)
            nc.sync.dma_start(w1t := sp.tile([128, 128], f32, tag="wtmp"),
                              wch[1, :, :]) if False else None
            w1t = sp.tile([128, 128], f32, tag="wtmp")
            nc.sync.dma_start(w1t[:], wch[1, :, :])
            nc.vector.tensor_copy(w1f[:], w1t[:])
            w2f = pp.tile([128, 128], f16)
            nc.vector.tensor_copy(w2f[:], w2t[:])
            wlt = pp.tile([128, 1], f32)
            nc.sync.dma_start(wlt[:], wlin[:, :])
            wlf = pp.tile([128, 1], f16)
            nc.vector.tensor_copy(wlf[:], wlt[:])
            bcht = pp.tile([128, 1], f32)
            nc.sync.dma_start(bcht[:], bch[:, :])

            # ---- deg / dis --------------------------------------------------
            deg = pp.tile([128, NW], f32)
            for t in range(NW):
                wt = sp.tile([128, PAD], f32, tag="wdeg")
                nc.sync.dma_start(wt[:], wpad[t, :, :])
                nc.vector.tensor_reduce(deg[:, t:t + 1], wt[:],
                                        axis=mybir.AxisListType.X, op=Alu.add)
            dmx = pp.tile([128, NW], f32)
            nc.vector.tensor_scalar(out=dmx[:], in0=deg[:], scalar1=1e-30,
                                    scalar2=None, op0=Alu.max)
            rec = pp.tile([128, NW], f32)
            nc.vector.reciprocal(rec[:], dmx[:])
            sq = pp.tile([128, NW], f32)
            nc.scalar.activation(sq[:], rec[:], Act.Sqrt)
            msk = pp.tile([128, NW], f32)
            nc.vector.tensor_scalar(out=msk[:], in0=deg[:], scalar1=0.0,
                                    scalar2=None, op0=Alu.is_gt)
            dis = pp.tile([128, NW], f32)
            nc.vector.tensor_tensor(out=dis[:], in0=sq[:], in1=msk[:],
                                    op=Alu.mult)
            mdis = pp.tile([128, NW], f32)
            nc.vector.tensor_scalar(out=mdis[:], in0=dis[:], scalar1=-1.0,
                                    scalar2=None, op0=Alu.mult)
            mdis2 = pp.tile([128, NW], f32)
            nc.vector.tensor_tensor(out=mdis2[:], in0=dis[:], in1=mdis[:],
                                    op=Alu.mult)
            m2x = pp.tile([128, NW], f32)
            nc.vector.tensor_scalar(out=m2x[:], in0=dis[:], scalar1=-2.0,
                                    scalar2=None, op0=Alu.mult)

            # ---- g1 = dis * x -> ag1_in; AllGather --------------------------
            for t in range(NW):
                xt = sp.tile([128, F], f32, tag="xprep")
                nc.sync.dma_start(xt[:], x32[t * 128:(t + 1) * 128, :])
                g1t = sp.tile([128, F], f16, tag="g1prep")
                nc.vector.tensor_scalar(out=g1t[:], in0=xt[:],
                                        scalar1=dis[:, t:t + 1], scalar2=None,
                                        op0=Alu.mult)
                nc.sync.dma_start(ag1_in[t * 128:(t + 1) * 128, :], g1t[:])
            nc.gpsimd.collective_compute(
                "AllGather", Alu.bypass, ins=[ag1_in[:, :]],
                outs=[g1_full[:, :]], replica_groups=rg)

            y_acc = pp.tile([128, NW * 128], f32)

            # ---- one SpMM pass over all edges -------------------------------
            def spmm(table):
                gathered = {}
                qrot = [0]

                def ensure(call):
                    if call in gathered:
                        return
                    cs, nch = call_meta[call]
                    it = sp.tile([128, GCH * 8], i16, tag="idx")
                    nc.sync.dma_start(it[:, :nch * 8], idxs[call, :, :nch * 8])
                    g = gp.tile([128, GCH * 128], f16, tag="g")
                    qq = 0
                    while qq < 3 and cs >= p["cbase"][qq + 1]:
                        qq += 1
                    nc.gpsimd.dma_gather(
                        out_ap=g[:, :nch * 128].rearrange(
                            "p (c f) -> p c f", f=F),
                        in_ap=table[qq * QT:(qq + 1) * QT, :],
                        idxs_ap=it[:, :nch * 8],
                        num_idxs=nch * 128, num_idxs_reg=nch * 128,
                        elem_size=F, single_packet=False,
                        queue_num=qrot[0] % 4)
                    qrot[0] += 1
                    gathered[call] = g

                # call -> (first chunk, count); chunk c lives in call
                c2call = np.empty(TOTCH, np.int64)
                c2slot = np.empty(TOTCH, np.int64)
                for i, (cs, n) in enumerate(call_meta):
                    c2call[cs:cs + n] = i
                    c2slot[cs:cs + n] = np.arange(n)
                ch = 0
                for q in range(4):
                    for wdx in range(NW):
                        kk = int(K[q][wdx])
                        ps = psA.tile([128, 128], f32, tag="ps")
                        for k in range(kk):
                            call = int(c2call[ch])
                            slot = int(c2slot[ch])
                            ensure(call)
                            oh = ohp.tile([128, 128], f16, tag="oh")
                            nc.vector.tensor_scalar(
                                out=oh[:], in0=iota_f[:],
                                scalar1=dstw_t[:, 2 * ch:2 * ch + 1],
                                scalar2=dstw_t[:, 2 * ch + 1:2 * ch + 2],
                                op0=Alu.is_equal, op1=Alu.mult)
                            nc.tensor.matmul(
                                out=ps[:], lhsT=oh[:],
                                rhs=gathered[call][:, slot * 128:(slot + 1) * 128],
                                start=(k == 0), stop=(k == kk - 1))
                            ch += 1
                        ysl = y_acc[:, wdx * 128:(wdx + 1) * 128]
                        if q == 0:
                            nc.vector.tensor_copy(ysl, ps[:])
                        else:
                            nc.vector.tensor_tensor(out=ysl, in0=ysl,
                                                    in1=ps[:], op=Alu.add)

            spmm(g1_full)
            for t in range(NW):
                ysl = y_acc[:, t * 128:(t + 1) * 128]
                t1 = sp.tile([128, F], f16, tag="tx1")
                nc.scalar.activation(t1[:], ysl, Act.Copy,
                                     scale=mdis[:, t:t + 1])
                nc.sync.dma_start(tx1s[t * 128:(t + 1) * 128, :], t1[:])
                g2t = sp.tile([128, F], f16, tag="g2e")
                nc.scalar.activation(g2t[:], ysl, Act.Copy,
                                     scale=mdis2[:, t:t + 1])
                nc.sync.dma_start(ag2_in[t * 128:(t + 1) * 128, :], g2t[:])
            nc.gpsimd.collective_compute(
                "AllGather", Alu.bypass, ins=[ag2_in[:, :]],
                outs=[g2_full[:, :]], replica_groups=rg)

            spmm(g2_full)
            for t in range(NW):
                s2t = sp.tile([128, F], f16, tag="s2e")
                nc.scalar.activation(s2t[:], y_acc[:, t * 128:(t + 1) * 128],
                                     Act.Copy, scale=m2x[:, t:t + 1])
                nc.sync.dma_start(s2s[t * 128:(t + 1) * 128, :], s2t[:])

            # ---- dense epilogue --------------------------------------------
            for t in range(NW):
                sl = slice(t * 128, (t + 1) * 128)
                xT = sp.tile([128, 128], f16, tag="xT")
                nc.sync.dma_start(xT[:], x16[sl, :], transpose=True)
                t1T = sp.tile([128, 128], f16, tag="t1T")
                nc.sync.dma_start(t1T[:], tx1s[sl, :], transpose=True)
                s2T = sp.tile([128, 128], f16, tag="s2T")
                nc.sync.dma_start(s2T[:], s2s[sl, :], transpose=True)
                po = psB.tile([128, 128], f32, tag="po")
                nc.tensor.matmul(out=po[:], lhsT=w02f[:], rhs=xT[:],
                                 start=True, stop=False)
                nc.tensor.matmul(out=po[:], lhsT=w1f[:], rhs=t1T[:],
                                 start=False, stop=False)
                nc.tensor.matmul(out=po[:], lhsT=w2f[:], rhs=s2T[:],
                                 start=False, stop=True)
                rl = sp.tile([128, 128], f16, tag="rl")
                nc.scalar.activation(rl[:], po[:], Act.Relu, bias=bcht[:])
                pf = psC.tile([128, 1], f32, tag="pf")
                nc.tensor.matmul(out=pf[:], lhsT=rl[:], rhs=wlf[:],
                                 start=True, stop=True)
                yt = sp.tile([128, 1], f32, tag="yt")
                nc.vector.tensor_scalar(out=yt[:], in0=pf[:],
                                        scalar1=float(b_lin_val), scalar2=None,
                                        op0=Alu.add)
                nc.sync.dma_start(out[sl, :], yt[:])
    nc.compile()
    return nc


def kernel(x, edge_index, edge_weight, W_cheb, b_cheb, W_lin, b_lin):
    x = np.asarray(x)
    n_cores = 8
    p, in_maps = _plan(x, np.asarray(edge_index), np.asarray(edge_weight),
                       n_cores)
    wch = np.asarray(W_cheb, np.float32)
    bch = np.asarray(b_cheb, np.float32).reshape(128, 1)
    wl = np.asarray(W_lin, np.float32).reshape(128, 1)
    blv = float(np.asarray(b_lin).reshape(-1)[0])
    for m in in_maps:
        m["wch"] = wch
        m["bch"] = bch
        m["wlin"] = wl
    nc = _build(p, blv)
    r = bass_utils.run_bass_kernel_spmd(
        nc, in_maps, core_ids=list(range(n_cores)), trace=TRACE[0])
    LAST_EXEC_NS[0] = r.exec_time_ns
    S_LOG, N = p["S_LOG"], p["N"]
    outs = [np.asarray(r.results[c]["out"])[:min(S_LOG, N - c * S_LOG)]
            for c in range(n_cores)]
    return np.concatenate(outs, axis=0).astype(np.float32)



# revision 4
# speedup vs baseline: 1.3416x; 1.3416x over previous
"""ChebyshevGCN (K=3) on 8 TRN2 NeuronCores — v2.

Design (dst-sharded SpMM via one-hot matmuls, SWDGE gather):
  - Nodes dst-sharded (12500/core, 98 windows of 128). Edge slots are padded
    per (quarter, window) to the max count over cores (SPMD shared structure)
    but NOT rounded to 128: gather chunks span window boundaries, with one
    one-hot matmul per (chunk, window) segment. ~212k slots/pass vs 262k for
    chunk-aligned padding.
  - Gather: SWDGE dma_gather (int16 idx, 4 sub-tables of 25088 rows, 4
    queues, prefetch depth 2). Desc-gen on GpSimd is the critical path
    (~7.3ns/row); everything else is hidden under it.
  - Scatter: one-hot [slot, dstoff] built on DVE (is_equal+mult vs iota),
    emitted one window ahead of the PSUM-reading accumulation ops so the
    in-order DVE queue never starves the PE.
  - Epilogue fused per window into pass-2 completion: y2 -> S2 (ACT scale) ->
    PE-transpose -> filter-major matmuls -> relu -> W_lin. xT/tx1T built the
    same way (PE transpose + DVE copy, no DMA transposes, no DRAM roundtrip).
"""
import sys
import numpy as np

if "/opt/trn_rl_repo" not in sys.path:
    sys.path.insert(0, "/opt/trn_rl_repo")

import concourse.bass as bass  # noqa: F401
import concourse.mybir as mybir
import concourse.tile as tile
from concourse import bacc, bass_utils

F = 128
GCH = 32          # chunks (of 128 slots) per dma_gather call
TRACE = [False]
LAST_EXEC_NS = [None]


def _ceil(a, b):
    return (a + b - 1) // b


def _plan(x, edge_index, edge_weight, n_cores=8):
    N = x.shape[0]
    S_LOG = _ceil(N, n_cores)                 # 12500
    SHARD = _ceil(S_LOG, 128) * 128           # 12544
    NW = SHARD // 128                         # 98
    NTAB = n_cores * SHARD                    # 100352
    QT = NTAB // 4                            # 25088
    assert QT <= 32768

    src = np.asarray(edge_index[0], dtype=np.int64)
    dst = np.asarray(edge_index[1], dtype=np.int64)
    wgt = np.asarray(edge_weight, dtype=np.float32)
    E = len(wgt)

    owner = dst // S_LOG
    dl = dst - owner * S_LOG
    srow = (src // S_LOG) * SHARD + (src % S_LOG)
    q_of = srow // QT
    qidx = (srow % QT).astype(np.int16)
    win = dl // 128
    doff = (dl % 128).astype(np.float32)
    run = q_of * NW + win                     # [E] run id 0..4*NW-1

    # per-core run counts -> shared M[q][w] (max over cores, >=1)
    cnts = np.zeros((n_cores, 4 * NW), np.int64)
    per_core_sel = []
    for c in range(n_cores):
        sel = np.nonzero(owner == c)[0]
        cnts[c] = np.bincount(run[sel], minlength=4 * NW)
        per_core_sel.append(sel)
    M = np.maximum(cnts.max(axis=0), 1)       # [4*NW] slots per (q,w)

    # slot layout: quarter streams chunk-padded; windows packed inside
    Mq = M.reshape(4, NW)
    Sq = Mq.sum(axis=1)                       # slots per quarter (unpadded)
    CQ = _ceil(Sq, 128)                       # chunks per quarter
    cbase = np.concatenate([[0], np.cumsum(CQ)])[:-1]
    TOTCH = int(CQ.sum())
    wbase = np.zeros((4, NW), np.int64)       # window slot base within quarter
    for q in range(4):
        wbase[q] = np.concatenate([[0], np.cumsum(Mq[q])])[:-1]
    slot_base = cbase[:, None] * 128 + wbase  # [4, NW] global slot base

    # segments: per chunk, overlapping windows (shared across cores)
    # seg = (q, chunk_global, w, start, stop) with start/stop = first/last
    # segment of window (q, w)
    segs = []
    seg_of_qw_last = {}
    for q in range(4):
        ends = wbase[q] + Mq[q]               # window end offsets in quarter
        for k in range(int(CQ[q])):
            a, b = k * 128, min((k + 1) * 128, int(Sq[q]))
            if a >= b:
                continue
            w0 = int(np.searchsorted(ends, a, side="right"))
            w = w0
            while w < NW and wbase[q][w] < b:
                segs.append([q, int(cbase[q]) + k, w,
                             wbase[q][w] <= a or wbase[q][w] == a,  # fixed below
                             False])
                w += 1
    # recompute start/stop properly
    first_seen = set()
    for i, s in enumerate(segs):
        key = (s[0], s[2])
        s[3] = key not in first_seen
        first_seen.add(key)
        seg_of_qw_last[key] = i
    for key, i in seg_of_qw_last.items():
        segs[i][4] = True
    NSEG = len(segs)

    # gather call metadata: per quarter, split CQ into GCH-chunk calls
    call_meta = []
    for q in range(4):
        left, cs = int(CQ[q]), int(cbase[q])
        while left > 0:
            n = min(GCH, left)
            call_meta.append((cs, n))
            cs += n
            left -= n
    NCALLS = len(call_meta)
    c2call = np.empty(TOTCH, np.int64)
    c2slot = np.empty(TOTCH, np.int64)
    for i, (cs, n) in enumerate(call_meta):
        c2call[cs:cs + n] = i
        c2slot[cs:cs + n] = np.arange(n)

    # out-degree padding for deg reduce
    deg_cnt = np.bincount(src, minlength=N)
    PAD = max(8, _ceil(int(deg_cnt.max()), 8) * 8)

    NS = TOTCH * 128
    in_maps = []
    for c in range(n_cores):
        sel = per_core_sel[c]
        rr = run[sel]
        order = np.argsort(rr, kind="stable")
        sel = sel[order]
        rr = rr[order]
        starts = np.concatenate([[0], np.cumsum(cnts[c])])[:-1]
        rank = np.arange(len(sel)) - starts[rr]
        slot = slot_base.reshape(-1)[rr] + rank
        qidx_s = np.zeros(NS, np.int16)
        doff_s = np.full(NS, 999.0, np.float32)
        wgt_s = np.zeros(NS, np.float32)
        qidx_s[slot] = qidx[sel]
        doff_s[slot] = doff[sel]
        wgt_s[slot] = wgt[sel]
        # dstw per segment: col0 = doff (masked 999 outside window), col1 = w
        dstw = np.empty((128, 2 * NSEG), np.float32)
        for i, (q, ch, w, _, _) in enumerate(segs):
            cs = ch * 128
            sl = np.arange(cs, cs + 128)
            a = slot_base[q][w]
            b = a + Mq[q][w]
            inw = (sl >= a) & (sl < b)
            dstw[:, 2 * i] = np.where(inw, doff_s[sl], 999.0)
            dstw[:, 2 * i + 1] = wgt_s[sl]
        idxs = np.zeros((NCALLS, 128, GCH * 8), np.int16)
        for i, (cs, n) in enumerate(call_meta):
            ids = qidx_s[cs * 128:(cs + n) * 128]
            wrap = ids.reshape(n * 8, 16).T           # [16, n*8]
            idxs[i, :, :n * 8] = np.tile(wrap, (8, 1))
        # wpad for deg (out-edges of own shard nodes)
        sel2 = np.nonzero(src // S_LOG == c)[0]
        loc = (src[sel2] - c * S_LOG).astype(np.int64)
        o2 = np.argsort(loc, kind="stable")
        sel2, loc = sel2[o2], loc[o2]
        c2 = np.bincount(loc, minlength=S_LOG)
        st2 = np.concatenate([[0], np.cumsum(c2)])[:-1]
        rk2 = np.arange(len(sel2)) - st2[loc]
        wpad = np.zeros((NW, 128, PAD), np.float32)
        wpad[loc // 128, loc % 128, rk2] = wgt[sel2]
        xs = np.zeros((SHARD, F), np.float32)
        n0, n1 = c * S_LOG, min((c + 1) * S_LOG, N)
        xs[: n1 - n0] = np.asarray(x[n0:n1], np.float32)
        in_maps.append({"x32": xs, "wpad": wpad, "dstw": dstw, "idxs": idxs})

    shape = dict(N=N, S_LOG=S_LOG, SHARD=SHARD, NTAB=NTAB, QT=QT, NW=NW,
                 PAD=PAD, TOTCH=TOTCH, NCALLS=NCALLS, NSEG=NSEG, segs=segs,
                 call_meta=call_meta, cbase=cbase, c2call=c2call,
                 c2slot=c2slot, n_cores=n_cores)
    return shape, in_maps


def _build(p, b_lin_val):
    n_cores, SHARD, NTAB, QT, NW, PAD, TOTCH, NCALLS, NSEG = (
        p["n_cores"], p["SHARD"], p["NTAB"], p["QT"], p["NW"], p["PAD"],
        p["TOTCH"], p["NCALLS"], p["NSEG"])
    segs, call_meta, cbase = p["segs"], p["call_meta"], p["cbase"]
    c2call, c2slot = p["c2call"], p["c2slot"]
    f32, f16, i16 = mybir.dt.float32, mybir.dt.float16, mybir.dt.int16
    Alu, Act = mybir.AluOpType, mybir.ActivationFunctionType

    nc = bacc.Bacc("TRN2", target_bir_lowering=False, debug=False,
                   num_devices=n_cores, num_swdge_queues=4)
    x32 = nc.dram_tensor("x32", [SHARD, F], f32, kind="ExternalInput")
    wpad = nc.dram_tensor("wpad", [NW, 128, PAD], f32, kind="ExternalInput")
    dstw = nc.dram_tensor("dstw", [128, 2 * NSEG], f32, kind="ExternalInput")
    idxs = nc.dram_tensor("idxs", [NCALLS, 128, GCH * 8], i16,
                          kind="ExternalInput")
    wch = nc.dram_tensor("wch", [3, 128, 128], f32, kind="ExternalInput")
    bch = nc.dram_tensor("bch", [128, 1], f32, kind="ExternalInput")
    wlin = nc.dram_tensor("wlin", [128, 1], f32, kind="ExternalInput")
    iden = nc.dram_tensor("iden", [128, 128], f16, kind="ExternalInput")
    iota = nc.dram_tensor("iota", [128, 128], f16, kind="ExternalInput")
    out = nc.dram_tensor("out", [SHARD, 1], f32, kind="ExternalOutput")

    ag1_in = nc.dram_tensor("ag1_in", [SHARD, F], f16, kind="Internal")
    g1_full = nc.dram_tensor("g1_full", [NTAB, F], f16, kind="Internal",
                             addr_space="Shared")
    ag2_in = nc.dram_tensor("ag2_in", [SHARD, F], f16, kind="Internal")
    g2_full = nc.dram_tensor("g2_full", [NTAB, F], f16, kind="Internal",
                             addr_space="Shared")
    rg = [list(range(n_cores))]

    # segments grouped by (q, w) for emission
    segs_by_qw = [[[] for _ in range(NW)] for _ in range(4)]
    for i, (q, ch, w, st, sp) in enumerate(segs):
        segs_by_qw[q][w].append((i, ch, st, sp))

    with tile.TileContext(nc) as tc:
        with tc.tile_pool(name="pp", bufs=1) as pp, \
             tc.tile_pool(name="sp", bufs=3) as sp, \
             tc.tile_pool(name="ip", bufs=6) as ipool, \
             tc.tile_pool(name="gst", bufs=4) as gp, \
             tc.tile_pool(name="oh", bufs=14) as ohp, \
             tc.tile_pool(name="psA", bufs=2, space="PSUM") as psA, \
             tc.tile_pool(name="psB", bufs=2, space="PSUM") as psB, \
             tc.tile_pool(name="psC", bufs=2, space="PSUM") as psC:

            # ---- consts ----------------------------------------------------
            dstw_t = pp.tile([128, 2 * NSEG], f32)
            nc.sync.dma_start(dstw_t[:], dstw[:, :])
            iota_f = pp.tile([128, 128], f16)
            nc.sync.dma_start(iota_f[:], iota[:, :])
            ident = pp.tile([128, 128], f16)
            nc.sync.dma_start(ident[:], iden[:, :])
            w0t = pp.tile([128, 128], f32)
            w2t = pp.tile([128, 128], f32)
            nc.sync.dma_start(w0t[:], wch[0, :, :])
            nc.sync.dma_start(w2t[:], wch[2, :, :])
            w02f = pp.tile([128, 128], f16)
            nc.vector.tensor_tensor(out=w02f[:], in0=w0t[:], in1=w2t[:],
                                    op=Alu.subtract)
            w1t = sp.tile([128, 128], f32, tag="wtmp")
            nc.sync.dma_start(w1t[:], wch[1, :, :])
            w1f = pp.tile([128, 128], f16)
            nc.vector.tensor_copy(w1f[:], w1t[:])
            w2f = pp.tile([128, 128], f16)
            nc.vector.tensor_copy(w2f[:], w2t[:])
            wlt = pp.tile([128, 1], f32)
            nc.sync.dma_start(wlt[:], wlin[:, :])
            wlf = pp.tile([128, 1], f16)
            nc.vector.tensor_copy(wlf[:], wlt[:])
            bcht = pp.tile([128, 1], f32)
            nc.sync.dma_start(bcht[:], bch[:, :])

            # ---- deg / dis -------------------------------------------------
            deg = pp.tile([128, NW], f32)
            for t in range(NW):
                wt = sp.tile([128, PAD], f32, tag="wdeg")
                nc.sync.dma_start(wt[:], wpad[t, :, :])
                nc.vector.tensor_reduce(deg[:, t:t + 1], wt[:],
                                        axis=mybir.AxisListType.X, op=Alu.add)
            dmx = pp.tile([128, NW], f32)
            nc.vector.tensor_scalar(out=dmx[:], in0=deg[:], scalar1=1e-30,
                                    scalar2=None, op0=Alu.max)
            rec = pp.tile([128, NW], f32)
            nc.vector.reciprocal(rec[:], dmx[:])
            sq = pp.tile([128, NW], f32)
            nc.scalar.activation(sq[:], rec[:], Act.Sqrt)
            msk = pp.tile([128, NW], f32)
            nc.vector.tensor_scalar(out=msk[:], in0=deg[:], scalar1=0.0,
                                    scalar2=None, op0=Alu.is_gt)
            dis = pp.tile([128, NW], f32)
            nc.vector.tensor_tensor(out=dis[:], in0=sq[:], in1=msk[:],
                                    op=Alu.mult)
            mdis = pp.tile([128, NW], f32)
            nc.vector.tensor_scalar(out=mdis[:], in0=dis[:], scalar1=-1.0,
                                    scalar2=None, op0=Alu.mult)
            mdis2 = pp.tile([128, NW], f32)
            nc.vector.tensor_tensor(out=mdis2[:], in0=dis[:], in1=mdis[:],
                                    op=Alu.mult)
            m2x = pp.tile([128, NW], f32)
            nc.vector.tensor_scalar(out=m2x[:], in0=dis[:], scalar1=-2.0,
                                    scalar2=None, op0=Alu.mult)

            # ---- xT tiles + g1 = dis*x -> ag1_in; AllGather ---------------
            xT = pp.tile([128, NW * 128], f16)
            tx1T = pp.tile([128, NW * 128], f16)
            for t in range(NW):
                xt32 = sp.tile([128, F], f32, tag="xprep")
                nc.sync.dma_start(xt32[:], x32[t * 128:(t + 1) * 128, :])
                x16t = sp.tile([128, F], f16, tag="x16p")
                nc.vector.tensor_copy(x16t[:], xt32[:])
                ptr = psB.tile([128, 128], f16, tag="ptr")
                nc.tensor.transpose(ptr[:], x16t[:], ident[:])
                nc.vector.tensor_copy(xT[:, t * 128:(t + 1) * 128], ptr[:])
                g1t = sp.tile([128, F], f16, tag="g1prep")
                nc.vector.tensor_scalar(out=g1t[:], in0=xt32[:],
                                        scalar1=dis[:, t:t + 1], scalar2=None,
                                        op0=Alu.mult)
                nc.sync.dma_start(ag1_in[t * 128:(t + 1) * 128, :], g1t[:])
            nc.gpsimd.collective_compute(
                "AllGather", Alu.bypass, ins=[ag1_in[:, :]],
                outs=[g1_full[:, :]], replica_groups=rg)

            y_acc = pp.tile([128, NW * 128], f32)

            # ---- SpMM pass over all edges ---------------------------------
            def spmm(table, finish_window):
                gathered = {}
                qrot = [0]

                def ensure(call):
                    if call in gathered or call >= NCALLS:
                        return
                    cs, nch = call_meta[call]
                    it = ipool.tile([128, GCH * 8], i16, tag="idx")
                    nc.sync.dma_start(it[:, :nch * 8], idxs[call, :, :nch * 8])
                    g = gp.tile([128, GCH * 128], f16, tag="g")
                    qq = 0
                    while qq < 3 and cs >= cbase[qq + 1]:
                        qq += 1
                    nc.gpsimd.dma_gather(
                        out_ap=g[:, :nch * 128].rearrange(
                            "p (c f) -> p c f", f=F),
                        in_ap=table[qq * QT:(qq + 1) * QT, :],
                        idxs_ap=it[:, :nch * 8],
                        num_idxs=nch * 128, num_idxs_reg=nch * 128,
                        elem_size=F, single_packet=False,
                        queue_num=qrot[0] % 4)
                    qrot[0] += 1
                    gathered[call] = g

                oh_tiles = {}

                def emit_onehots(q, w):
                    for (i, ch, st, sp_) in segs_by_qw[q][w]:
                        oh = ohp.tile([128, 128], f16, tag="oh")
                        nc.vector.tensor_scalar(
                            out=oh[:], in0=iota_f[:],
                            scalar1=dstw_t[:, 2 * i:2 * i + 1],
                            scalar2=dstw_t[:, 2 * i + 1:2 * i + 2],
                            op0=Alu.is_equal, op1=Alu.mult)
                        oh_tiles[i] = oh

                def emit_window(q, w):
                    ps = psA.tile([128, 128], f32, tag="ps")
                    for (i, ch, st, sp_) in segs_by_qw[q][w]:
                        call = int(c2call[ch])
                        slot = int(c2slot[ch])
                        ensure(call)
                        ensure(call + 1)
                        ensure(call + 2)
                        nc.tensor.matmul(
                            out=ps[:], lhsT=oh_tiles.pop(i)[:],
                            rhs=gathered[call][:, slot * 128:(slot + 1) * 128],
                            start=st, stop=sp_)
                    ysl = y_acc[:, w * 128:(w + 1) * 128]
                    if q == 0:
                        nc.vector.tensor_copy(ysl, ps[:])
                    elif q < 3:
                        nc.vector.tensor_tensor(out=ysl, in0=ysl, in1=ps[:],
                                                op=Alu.add)
                    else:
                        finish_window(w, ps)

                for q in range(4):
                    emit_onehots(q, 0)
                    for w in range(1, NW):
                        emit_onehots(q, w)
                        emit_window(q, w - 1)
                    emit_window(q, NW - 1)

            # ---- pass 1: finish -> tx1T tiles + ag2_in --------------------
            def finish1(w, ps):
                ysl = y_acc[:, w * 128:(w + 1) * 128]
                y1 = sp.tile([128, 128], f32, tag="y1w")
                nc.vector.tensor_tensor(out=y1[:], in0=ysl, in1=ps[:],
                                        op=Alu.add)
                t1 = sp.tile([128, F], f16, tag="tx1")
                nc.scalar.activation(t1[:], y1[:], Act.Copy,
                                     scale=mdis[:, w:w + 1])
                ptr = psB.tile([128, 128], f16, tag="ptr")
                nc.tensor.transpose(ptr[:], t1[:], ident[:])
                nc.vector.tensor_copy(tx1T[:, w * 128:(w + 1) * 128], ptr[:])
                g2t = sp.tile([128, F], f16, tag="g2e")
                nc.scalar.activation(g2t[:], y1[:], Act.Copy,
                                     scale=mdis2[:, w:w + 1])
                nc.sync.dma_start(ag2_in[w * 128:(w + 1) * 128, :], g2t[:])

            spmm(g1_full, finish1)
            nc.gpsimd.collective_compute(
                "AllGather", Alu.bypass, ins=[ag2_in[:, :]],
                outs=[g2_full[:, :]], replica_groups=rg)

            # ---- pass 2: finish -> fused epilogue per window --------------
            def finish2(w, ps):
                ysl = y_acc[:, w * 128:(w + 1) * 128]
                y2 = sp.tile([128, 128], f32, tag="y2w")
                nc.vector.tensor_tensor(out=y2[:], in0=ysl, in1=ps[:],
                                        op=Alu.add)
                s2 = sp.tile([128, F], f16, tag="s2e")
                nc.scalar.activation(s2[:], y2[:], Act.Copy,
                                     scale=m2x[:, w:w + 1])
                ptr = psB.tile([128, 128], f16, tag="ptr")
                nc.tensor.transpose(ptr[:], s2[:], ident[:])
                s2T = sp.tile([128, 128], f16, tag="s2T")
                nc.vector.tensor_copy(s2T[:], ptr[:])
                sl = slice(w * 128, (w + 1) * 128)
                po = psC.tile([128, 128], f32, tag="po")
                nc.tensor.matmul(out=po[:], lhsT=w02f[:], rhs=xT[:, sl],
                                 start=True, stop=False)
                nc.tensor.matmul(out=po[:], lhsT=w1f[:], rhs=tx1T[:, sl],
                                 start=False, stop=False)
                nc.tensor.matmul(out=po[:], lhsT=w2f[:], rhs=s2T[:],
                                 start=False, stop=True)
                rl = sp.tile([128, 128], f16, tag="rl")
                nc.scalar.activation(rl[:], po[:], Act.Relu, bias=bcht[:])
                pf = psC.tile([128, 1], f32, tag="pf")
                nc.tensor.matmul(out=pf[:], lhsT=rl[:], rhs=wlf[:],
                                 start=True, stop=True)
                yt = sp.tile([128, 1], f32, tag="yt")
                nc.vector.tensor_scalar(out=yt[:], in0=pf[:],
                                        scalar1=float(b_lin_val), scalar2=None,
                                        op0=Alu.add)
                nc.sync.dma_start(out[sl, :], yt[:])

            spmm(g2_full, finish2)
    nc.compile()
    return nc


def kernel(x, edge_index, edge_weight, W_cheb, b_cheb, W_lin, b_lin):
    x = np.asarray(x)
    n_cores = 8
    p, in_maps = _plan(x, np.asarray(edge_index), np.asarray(edge_weight),
                       n_cores)
    wch = np.asarray(W_cheb, np.float32)
    bch = np.asarray(b_cheb, np.float32).reshape(128, 1)
    wl = np.asarray(W_lin, np.float32).reshape(128, 1)
    blv = float(np.asarray(b_lin).reshape(-1)[0])
    iden = np.eye(128, dtype=np.float16)
    iota = np.tile(np.arange(128, dtype=np.float16), (128, 1))
    for m in in_maps:
        m["wch"] = wch
        m["bch"] = bch
        m["wlin"] = wl
        m["iden"] = iden
        m["iota"] = iota
    nc = _build(p, blv)
    r = bass_utils.run_bass_kernel_spmd(
        nc, in_maps, core_ids=list(range(n_cores)), trace=TRACE[0])
    LAST_EXEC_NS[0] = r.exec_time_ns
    S_LOG, N = p["S_LOG"], p["N"]
    outs = [np.asarray(r.results[c]["out"])[:min(S_LOG, N - c * S_LOG)]
            for c in range(n_cores)]
    return np.concatenate(outs, axis=0).astype(np.float32)
